# revision 28
# baseline (speedup 1.0000x reference)
"""Trainium2 Bass kernel for nn_AudioMamba1Model (L=1 Mamba => pure per-row pipeline).

Math (per row of x[36]):
  xc = diag(cw)@(in_proj[:24]@(f_in@x+b1)) + cb ; xi' = silu(xc)
  z  = in_proj[24:]@(f_in@x+b1)               ; sz  = silu(z)
  q  = x_proj@xi' ; dt = softplus(dtw*q[0]+dtb); s = q[1:5]@q[5:9]
  y  = xi'*(dt*s + Dp)*sz ; probs = softmax(f_out@(out_proj@y)+b5)

Device strategy: 8-way data parallel over rows. Per core, feature-major layout
with G=3 row-groups packed into partitions; all linear maps are PE matmuls with
host-fused block-diagonal fp16 weights; silu via tanh identity (2*silu(x) =
x*(1+tanh(x/2))), softplus via exp/ln, softmax via exp + ones-matmul sums +
fast reciprocal + ones-matmul broadcast. Host does transposes/padding/casts.
"""
import numpy as np

B = 524288
NCORES = 8
RPC = B // NCORES            # 65536 rows per core
G = 3
NCHUNK = 512                 # matmul moving size (columns per chunk)
SUPER = G * NCHUNK           # rows per chunk
NSB = (RPC + SUPER - 1) // SUPER   # 43 chunks
RPAD = NSB * SUPER           # 66048 padded rows per core
NCOLS = RPAD // G            # 22016 columns per core

_PROGRAM = None
_RUN_KW = {}
_LAST_RESULT = None


def _blockdiag(w, g=G):
    """w:[k,m] -> block-diagonal [g*k, g*m]."""
    k, m = w.shape
    out = np.zeros((g * k, g * m), np.float32)
    for i in range(g):
        out[i * k:(i + 1) * k, i * m:(i + 1) * m] = w
    return out


def _fuse_weights(f_in_w, f_in_b, f_out_w, f_out_b, in_proj_w, conv_w, conv_b,
                  x_proj_w, dt_proj_w, dt_proj_b, A_log, Dp, out_proj_w):
    A = in_proj_w @ f_in_w                       # [48,36]
    bA = in_proj_w @ f_in_b                      # [48]
    cw = conv_w[:, 0, 1]
    A_xc = cw[:, None] * A[:24]; b_xc = cw * bA[:24] + conv_b
    A_z = A[24:]; b_z = bA[24:]
    W3 = x_proj_w
    W3dt = np.outer(dt_proj_w[:, 0], W3[0])      # [24,24]
    W3P = 0.5 * (W3[1:5] + W3[5:9])
    W3M = 0.5 * (W3[1:5] - W3[5:9])
    W3f = 0.5 * np.concatenate([W3dt, W3P, W3M], 0)   # [32,24]; 0.5 for xi'_m=2silu
    W54 = 0.25 * (f_out_w @ out_proj_w)          # [32,24]; 0.25 for xi'_m*sz_m=4*

    # lhsT matrices (stationary operands), fp16
    # L_xc/L_z: [109, 72]: x rows g*36+i, bias row 108; out g*24+d
    L_xc = np.zeros((109, 72), np.float32)
    L_z = np.zeros((109, 72), np.float32)
    L_xc[:108, :] = _blockdiag(A_xc.T)           # A_xc.T: [36,24]
    L_z[:108, :] = _blockdiag(A_z.T)
    for g in range(G):
        L_xc[108, g * 24:(g + 1) * 24] = b_xc
        L_z[108, g * 24:(g + 1) * 24] = b_z
    # L_q: [72, 96]: in g*24+i; out: dt at g*24+d (0..71), P at 72+g*4+n, M at 84+g*4+n
    L_q = np.zeros((72, 96), np.float32)
    L_q[:, :72] = _blockdiag(W3dt.T * 0.5)
    for g in range(G):
        L_q[g * 24:(g + 1) * 24, 72 + g * 4:76 + g * 4] = 0.5 * W3P.T
        L_q[g * 24:(g + 1) * 24, 84 + g * 4:88 + g * 4] = 0.5 * W3M.T
    # L_s: [24, 72]: sq rows: P g*4+n (0..11), M at 12+g*4+n; out s at g*24+d
    L_s = np.zeros((24, 72), np.float32)
    for g in range(G):
        L_s[g * 4:(g + 1) * 4, g * 24:(g + 1) * 24] = 1.0
        L_s[12 + g * 4:12 + (g + 1) * 4, g * 24:(g + 1) * 24] = -1.0
    # L_o: [72, 96] blockdiag W54.T
    L_o = _blockdiag(W54.T)
    # L_sum96: [96, 96] block all-ones: sums_b = L_sum96 @ e32 (broadcast sums)
    L_sum96 = np.zeros((96, 96), np.float32)
    for g in range(G):
        L_sum96[g * 32:(g + 1) * 32, g * 32:(g + 1) * 32] = 1.0
    # bias / scalar vectors (fp32 [P,1])
    dtb_t = np.tile(dt_proj_b, G)[:, None].astype(np.float32)        # [72,1]
    b5_t = np.tile(f_out_b, G)[:, None].astype(np.float32)           # [96,1]
    Dp_t = np.tile(Dp, G)[:, None].astype(np.float32)                # [72,1]
    f16 = np.float16
    return dict(Lxc=L_xc.astype(f16), Lz=L_z.astype(f16),
                Lqd=np.ascontiguousarray(L_q[:, 0:72]).astype(f16),
                Lqp=np.ascontiguousarray(L_q[:, 72:96]).astype(f16),
                Ls=L_s.astype(f16), Lo=L_o.astype(f16),
                Lsum=L_sum96.astype(f16), dtb=dtb_t, b5t=b5_t, Dpt=Dp_t)


def _build_program():
    import concourse.bass as bass
    import concourse.bacc as bacc
    import concourse.mybir as mybir
    from concourse.tile import TileContext
    dt = mybir.dt
    AF = mybir.ActivationFunctionType
    ALU = mybir.AluOpType
    f16, f32 = dt.float16, dt.float32

    nc = bacc.Bacc()
    xT = nc.dram_tensor("xT", [109, NCOLS], f16, kind="ExternalInput")
    w_dram = {}
    for name, shape in [("Lxc", [109, 72]), ("Lz", [109, 72]), ("Lqd", [72, 72]), ("Lqp", [72, 24]),
                        ("Ls", [24, 72]), ("Lo", [72, 96]), ("Lsum", [96, 96])]:
        w_dram[name] = nc.dram_tensor(name, shape, f16, kind="ExternalInput")
    for name, shape in [("dtb", [72, 1]), ("b5t", [96, 1]), ("Dpt", [72, 1])]:
        w_dram[name] = nc.dram_tensor(name, shape, f32, kind="ExternalInput")
    outT = nc.dram_tensor("outT", [96, NCOLS], f16, kind="ExternalOutput")

    with TileContext(nc) as tc:
        with tc.tile_pool(name="wp", bufs=1) as wp, \
             tc.tile_pool(name="persist", bufs=1) as pp, \
             tc.tile_pool(name="wk", bufs=2) as wk, \
             tc.tile_pool(name="psum", bufs=2, space="PSUM") as ps:
            w = {}
            for name, shape, dty in [("Lxc", [109, 72], f16), ("Lz", [109, 72], f16),
                                     ("Lqd", [72, 72], f16), ("Lqp", [72, 24], f16),
                                     ("Ls", [24, 72], f16),
                                     ("Lo", [72, 96], f16), ("Lsum", [96, 96], f16),
                                     ("dtb", [72, 1], f32),
                                     ("b5t", [96, 1], f32), ("Dpt", [72, 1], f32)]:
                w[name] = wp.tile(shape, dty, tag=name, name="w_"+name)
                nc.sync.dma_start(w[name][:, :], w_dram[name][:, :])

            xisz_all = pp.tile([72, 2 * NCOLS], f16, tag="xisz_all")
            xi_all = xisz_all[:, 0:NCOLS]
            sz_all = xisz_all[:, NCOLS:2 * NCOLS]
            ed_all = pp.tile([72, NCOLS], f16, tag="ed_all")
            sq_all = pp.tile([24, NCOLS], f16, tag="sq_all")

            # ---- Phase 1: table set exp_and_others (Tanh, Exp, Square) ----
            for c in range(NSB):
                sl = slice(c * NCHUNK, (c + 1) * NCHUNK)
                xt = wk.tile([109, NCHUNK], f16, tag="xt", bufs=8)
                nc.sync.dma_start(xt[:, :], xT[:, sl])
                xc = ps.tile([72, NCHUNK], f32, tag="pA")
                nc.tensor.matmul(xc[:, :], w["Lxc"][:, :], xt[:, :], start=True, stop=True)
                zc = ps.tile([72, NCHUNK], f32, tag="pB")
                nc.tensor.matmul(zc[:, :], w["Lz"][:, :], xt[:, :], start=True, stop=True)
                t1 = wk.tile([72, NCHUNK], f16, tag="t1", bufs=3)
                nc.scalar.activation(t1[:, :], xc[:, :], AF.Tanh, bias=0.0, scale=0.5)
                nc.vector.scalar_tensor_tensor(
                    xi_all[:, sl], t1[:, :], 1.0, xc[:, :], op0=ALU.add, op1=ALU.mult)
                t2 = wk.tile([72, NCHUNK], f16, tag="t2", bufs=3)
                nc.scalar.activation(t2[:, :], zc[:, :], AF.Tanh, bias=0.0, scale=0.5)
                nc.vector.scalar_tensor_tensor(
                    sz_all[:, sl], t2[:, :], 1.0, zc[:, :], op0=ALU.add, op1=ALU.mult)
                qd = ps.tile([72, NCHUNK], f32, tag="pC")
                nc.tensor.matmul(qd[:, :], w["Lqd"][:, :], xi_all[:, sl], start=True, stop=True)
                qp = ps.tile([24, NCHUNK], f32, tag="pD")
                nc.tensor.matmul(qp[:, :], w["Lqp"][:, :], xi_all[:, sl], start=True, stop=True)
                nc.scalar.activation(ed_all[:, sl], qd[:, :], AF.Exp,
                                     bias=w["dtb"][:, :], scale=1.0)
                nc.scalar.activation(sq_all[:, sl], qp[:, :], AF.Square)

            tc.strict_bb_all_engine_barrier()
            # ---- Phase 2a: softplus tail -- Ln only (set natural_log) ----
            NSLC = 8
            step = (NCOLS + NSLC - 1) // NSLC
            for k in range(NSLC):
                sl2 = slice(k * step, min((k + 1) * step, NCOLS))
                nc.vector.tensor_scalar(ed_all[:, sl2], ed_all[:, sl2], 1.0, None, op0=ALU.add)
                nc.scalar.activation(ed_all[:, sl2], ed_all[:, sl2], AF.Ln)

            tc.strict_bb_all_engine_barrier()
            # ---- Phase 2b: everything else -- Exp only (set exp_and_others) ----
            for c in range(NSB):
                sl = slice(c * NCHUNK, (c + 1) * NCHUNK)
                dtt = ed_all[:, sl]
                sb = ps.tile([72, NCHUNK], f32, tag="pA")
                nc.tensor.matmul(sb[:, :], w["Ls"][:, :], sq_all[:, sl], start=True, stop=True)
                u = wk.tile([72, NCHUNK], f16, tag="u")
                # u = (dt * sb) then + Dp ; STT: (dt mult-bypass?)  -> use two ops
                nc.vector.scalar_tensor_tensor(
                    u[:, :], dtt, 0.0, sb[:, :], op0=ALU.add, op1=ALU.mult)
                nc.vector.tensor_scalar(u[:, :], u[:, :], w["Dpt"][:, :], None, op0=ALU.add)
                u2 = u
                v = wk.tile([72, NCHUNK], f16, tag="v")
                nc.vector.tensor_tensor(v[:, :], xi_all[:, sl], sz_all[:, sl], op=ALU.mult)
                y2 = wk.tile([72, NCHUNK], f16, tag="y2")
                nc.vector.tensor_tensor(y2[:, :], v[:, :], u2[:, :], op=ALU.mult)
                o32 = ps.tile([96, NCHUNK], f32, tag="pC")
                nc.tensor.matmul(o32[:, :], w["Lo"][:, :], y2[:, :], start=True, stop=True)
                e32 = wk.tile([96, NCHUNK], f16, tag="e32")
                nc.scalar.activation(e32[:, :], o32[:, :], AF.Exp, bias=w["b5t"][:, :], scale=1.0)
                sums_b = ps.tile([96, NCHUNK], f32, tag="pB")
                nc.tensor.matmul(sums_b[:, :], w["Lsum"][:, :], e32[:, :], start=True, stop=True)
                rb = wk.tile([96, NCHUNK], f32, tag="rb96", bufs=2)
                nc.vector.reciprocal_approx_fast(rb[:, :], sums_b[:, :])
                if c % 4 == 0:
                    nbs = min(4, NSB - c)
                    pr_big = wk.tile([96, nbs * NCHUNK], f16, tag="pr", bufs=2,
                                     name=f"pr_big_{c}")
                pr = pr_big[:, (c % 4) * NCHUNK:(c % 4 + 1) * NCHUNK]
                nc.vector.tensor_tensor(pr[:, :], e32[:, :], rb[:, :], op=ALU.mult)
                if c % 4 == nbs - 1:
                    c0 = c - (c % 4)
                    nc.sync.dma_start(
                        outT[:, c0 * NCHUNK:(c0 + nbs) * NCHUNK], pr_big[:, :])
    nc.compile()
    return nc


def _get_program():
    global _PROGRAM
    if _PROGRAM is None:
        _PROGRAM = _build_program()
    return _PROGRAM


def kernel(**inputs) -> np.ndarray:
    from concourse.bass_utils import run_bass_kernel_spmd

    np_inputs = {k: np.asarray(v, np.float32) for k, v in inputs.items()}
    x = np_inputs.pop("x")
    weights = _fuse_weights(**np_inputs)

    in_maps = []
    for c in range(NCORES):
        xc = x[c * RPC:(c + 1) * RPC]
        xp = np.zeros((RPAD, 36), np.float32)
        xp[:RPC] = xc
        # row = g*NCOLS + n  ->  [G, NCOLS, 36] -> [G, 36, NCOLS] -> [108, NCOLS]
        xt = np.ascontiguousarray(
            xp.reshape(G, NCOLS, 36).transpose(0, 2, 1).reshape(108, NCOLS))
        xfull = np.ones((109, NCOLS), np.float32)
        xfull[:108] = xt
        in_maps.append({"xT": xfull.astype(np.float16), **weights})

    nc = _get_program()
    res = run_bass_kernel_spmd(nc, in_maps, core_ids=list(range(NCORES)), **_RUN_KW)
    global _LAST_RESULT
    _LAST_RESULT = res
    if getattr(res, "exec_time_ns", None):
        print(f"HW exec time: {res.exec_time_ns} ns")
    outs = []
    for c in range(NCORES):
        oT = np.asarray(res.results[c]["outT"], np.float32)   # [96, NCOLS]
        # partition g*32+f, col n -> row g*NCOLS+n, feature f
        o = oT.reshape(G, 32, NCOLS).transpose(0, 2, 1).reshape(RPAD, 32)
        outs.append(o[:RPC])
    return np.concatenate(outs, 0).astype(np.float32)


if __name__ == "__main__":
    nc = _build_program()
    print("program built OK")


# revision 34
# speedup vs baseline: 1.2944x; 1.2944x over previous
"""Trainium2 Bass kernel for nn_AudioMamba1Model (L=1 Mamba => pure per-row pipeline).

Math (per row of x[36]):
  xc = diag(cw)@(in_proj[:24]@(f_in@x+b1)) + cb ; xi' = silu(xc)
  z  = in_proj[24:]@(f_in@x+b1)               ; sz  = silu(z)
  q  = x_proj@xi' ; dt = softplus(dtw*q[0]+dtb); s = q[1:5]@q[5:9]
  y  = xi'*(dt*s + Dp)*sz ; probs = softmax(f_out@(out_proj@y)+b5)

Device strategy: 8-way data parallel over rows. Per core, feature-major layout
with G=3 row-groups packed into partitions; all linear maps are PE matmuls with
host-fused block-diagonal fp16 weights; silu via tanh identity (2*silu(x) =
x*(1+tanh(x/2))), softplus via exp/ln, softmax via exp + ones-matmul sums +
fast reciprocal + ones-matmul broadcast. Host does transposes/padding/casts.
"""
import numpy as np

B = 524288
NCORES = 8
RPC = B // NCORES            # 65536 rows per core
G = 3
NCHUNK = 512                 # matmul moving size (columns per chunk)
SUPER = G * NCHUNK           # rows per chunk
NSB = (RPC + SUPER - 1) // SUPER   # 43 chunks
RPAD = NSB * SUPER           # 66048 padded rows per core
NCOLS = RPAD // G            # 22016 columns per core

_PROGRAM = None
_RUN_KW = {}
_LAST_RESULT = None


def _blockdiag(w, g=G):
    """w:[k,m] -> block-diagonal [g*k, g*m]."""
    k, m = w.shape
    out = np.zeros((g * k, g * m), np.float32)
    for i in range(g):
        out[i * k:(i + 1) * k, i * m:(i + 1) * m] = w
    return out


def _fuse_weights(f_in_w, f_in_b, f_out_w, f_out_b, in_proj_w, conv_w, conv_b,
                  x_proj_w, dt_proj_w, dt_proj_b, A_log, Dp, out_proj_w):
    A = in_proj_w @ f_in_w                       # [48,36]
    bA = in_proj_w @ f_in_b                      # [48]
    cw = conv_w[:, 0, 1]
    A_xc = cw[:, None] * A[:24]; b_xc = cw * bA[:24] + conv_b
    A_z = A[24:]; b_z = bA[24:]
    W3 = x_proj_w
    W3dt = np.outer(dt_proj_w[:, 0], W3[0])      # [24,24]
    W3P = 0.5 * (W3[1:5] + W3[5:9])
    W3M = 0.5 * (W3[1:5] - W3[5:9])
    W3f = 0.5 * np.concatenate([W3dt, W3P, W3M], 0)   # [32,24]; 0.5 for xi'_m=2silu
    W54 = 0.25 * (f_out_w @ out_proj_w)          # [32,24]; 0.25 for xi'_m*sz_m=4*

    # lhsT matrices (stationary operands), fp16
    # L_xc/L_z: [109, 72]: x rows g*36+i, bias row 108; out g*24+d
    L_xc = np.zeros((109, 72), np.float32)
    L_z = np.zeros((109, 72), np.float32)
    L_xc[:108, :] = _blockdiag(A_xc.T)           # A_xc.T: [36,24]
    L_z[:108, :] = _blockdiag(A_z.T)
    for g in range(G):
        L_xc[108, g * 24:(g + 1) * 24] = b_xc
        L_z[108, g * 24:(g + 1) * 24] = b_z
    # L_q: [72, 96]: in g*24+i; out: dt at g*24+d (0..71), P at 72+g*4+n, M at 84+g*4+n
    L_q = np.zeros((72, 96), np.float32)
    L_q[:, :72] = _blockdiag(W3dt.T * 0.5)
    for g in range(G):
        L_q[g * 24:(g + 1) * 24, 72 + g * 4:76 + g * 4] = 0.5 * W3P.T
        L_q[g * 24:(g + 1) * 24, 84 + g * 4:88 + g * 4] = 0.5 * W3M.T
    # L_s: [24, 72]: sq rows: P g*4+n (0..11), M at 12+g*4+n; out s at g*24+d
    L_s = np.zeros((24, 72), np.float32)
    for g in range(G):
        L_s[g * 4:(g + 1) * 4, g * 24:(g + 1) * 24] = 1.0
        L_s[12 + g * 4:12 + (g + 1) * 4, g * 24:(g + 1) * 24] = -1.0
    # L_o: [72, 96] blockdiag W54.T ; L_oD folds the +Dp term of
    # y2 = v*(dt*s) + v*Dp into a second accumulating matmul
    L_o = _blockdiag(W54.T)
    L_oD = _blockdiag((W54 * Dp[None, :]).T)
    # L_sum96: [96, 96] block all-ones: sums_b = L_sum96 @ e32 (broadcast sums)
    L_sum96 = np.zeros((96, 96), np.float32)
    for g in range(G):
        L_sum96[g * 32:(g + 1) * 32, g * 32:(g + 1) * 32] = 1.0
    # bias / scalar vectors (fp32 [P,1])
    dtb_t = np.tile(dt_proj_b, G)[:, None].astype(np.float32)        # [72,1]
    b5_t = np.tile(f_out_b, G)[:, None].astype(np.float32)           # [96,1]
    Dp_t = np.tile(Dp, G)[:, None].astype(np.float32)                # [72,1]
    f16 = np.float16
    return dict(Lxc=L_xc.astype(f16), Lz=L_z.astype(f16),
                Lqd=np.ascontiguousarray(L_q[:, 0:72]).astype(f16),
                Lqp=np.ascontiguousarray(L_q[:, 72:96]).astype(f16),
                Ls=L_s.astype(f16), Lo=L_o.astype(f16), LoD=L_oD.astype(f16),
                Lsum=L_sum96.astype(f16), dtb=dtb_t, b5t=b5_t)


def _build_program():
    import concourse.bass as bass
    import concourse.bacc as bacc
    import concourse.mybir as mybir
    from concourse.tile import TileContext
    dt = mybir.dt
    AF = mybir.ActivationFunctionType
    ALU = mybir.AluOpType
    f16, f32 = dt.float16, dt.float32

    nc = bacc.Bacc()
    xT = nc.dram_tensor("xT", [109, NCOLS], f16, kind="ExternalInput")
    w_dram = {}
    for name, shape in [("Lxc", [109, 72]), ("Lz", [109, 72]), ("Lqd", [72, 72]), ("Lqp", [72, 24]),
                        ("Ls", [24, 72]), ("Lo", [72, 96]), ("LoD", [72, 96]), ("Lsum", [96, 96])]:
        w_dram[name] = nc.dram_tensor(name, shape, f16, kind="ExternalInput")
    for name, shape in [("dtb", [72, 1]), ("b5t", [96, 1])]:
        w_dram[name] = nc.dram_tensor(name, shape, f32, kind="ExternalInput")
    outT = nc.dram_tensor("outT", [96, NCOLS], f16, kind="ExternalOutput")

    with TileContext(nc) as tc:
        with tc.tile_pool(name="wp", bufs=1) as wp, \
             tc.tile_pool(name="persist", bufs=1) as pp, \
             tc.tile_pool(name="wk", bufs=2) as wk, \
             tc.tile_pool(name="psum", bufs=2, space="PSUM") as ps:
            w = {}
            for name, shape, dty in [("Lxc", [109, 72], f16), ("Lz", [109, 72], f16),
                                     ("Lqd", [72, 72], f16), ("Lqp", [72, 24], f16),
                                     ("Ls", [24, 72], f16),
                                     ("Lo", [72, 96], f16), ("LoD", [72, 96], f16),
                                     ("Lsum", [96, 96], f16), ("dtb", [72, 1], f32),
                                     ("b5t", [96, 1], f32)]:
                w[name] = wp.tile(shape, dty, tag=name, name="w_"+name)
                nc.sync.dma_start(w[name][:, :], w_dram[name][:, :])

            xisz_all = pp.tile([72, 2 * NCOLS], f16, tag="xisz_all")
            xi_all = xisz_all[:, 0:NCOLS]
            sz_all = xisz_all[:, NCOLS:2 * NCOLS]
            ed_all = pp.tile([72, NCOLS], f16, tag="ed_all")
            sq_all = pp.tile([24, NCOLS], f16, tag="sq_all")

            # ---- Phase 1: table set exp_and_others (Tanh, Exp, Square) ----
            for c in range(NSB):
                sl = slice(c * NCHUNK, (c + 1) * NCHUNK)
                xt = wk.tile([109, NCHUNK], f16, tag="xt", bufs=6)
                nc.sync.dma_start(xt[:, :], xT[:, sl])
                xcz = ps.tile([72, 2 * NCHUNK], f32, tag="pA")
                nc.tensor.matmul(xcz[:, 0:NCHUNK], w["Lxc"][:, :], xt[:, :], start=True, stop=True)
                nc.tensor.matmul(xcz[:, NCHUNK:2 * NCHUNK], w["Lz"][:, :], xt[:, :], start=True, stop=True)
                t1 = wk.tile([72, 2 * NCHUNK], f16, tag="t1", bufs=3)
                nc.scalar.activation(t1[:, :], xcz[:, :], AF.Tanh, bias=0.0, scale=0.5)
                xisz_out = xisz_all.rearrange("p (a n) -> p a n", a=2)[:, :, sl]
                nc.vector.scalar_tensor_tensor(
                    xisz_out, t1[:, :], 1.0, xcz[:, :], op0=ALU.add, op1=ALU.mult)
                qd = ps.tile([72, NCHUNK], f32, tag="pC")
                nc.tensor.matmul(qd[:, :], w["Lqd"][:, :], xi_all[:, sl], start=True, stop=True)
                qp = ps.tile([24, NCHUNK], f32, tag="pB")
                nc.tensor.matmul(qp[:, :], w["Lqp"][:, :], xi_all[:, sl], start=True, stop=True)
                nc.scalar.activation(ed_all[:, sl], qd[:, :], AF.Exp,
                                     bias=w["dtb"][:, :], scale=1.0)
                qp16 = wk.tile([24, NCHUNK], f16, tag="qp16")
                nc.vector.tensor_copy(qp16[:, :], qp[:, :])
                nc.gpsimd.tensor_tensor(sq_all[:, sl], qp16[:, :], qp16[:, :], op=ALU.mult)

            tc.strict_bb_all_engine_barrier()
            # ---- Phase 2: Ln + Exp, pinned set natural_log_exp_and_others ----
            # Explicit table load so the greedy resolver doesn't ping-pong
            # between exp_and_others (no Ln) and natural_log (no Exp).
            from concourse.hw_specs import get_activation_tables
            set_names = list(get_activation_tables(nc.m.arch).keys())
            nle_id = set_names.index("natural_log_exp_and_others")
            nc.scalar.add_instruction(mybir.InstLoadActFuncSet(
                name=nc.get_next_instruction_name(), ins=[], outs=[],
                act_func_set_id=nle_id))
            for c in range(NSB):
                sl = slice(c * NCHUNK, (c + 1) * NCHUNK)
                nc.scalar.activation(ed_all[:, sl], ed_all[:, sl], AF.Ln, bias=1.0)
                dtt = ed_all[:, sl]
                sb = ps.tile([72, NCHUNK], f32, tag="pA")
                nc.tensor.matmul(sb[:, :], w["Ls"][:, :], sq_all[:, sl], start=True, stop=True)
                u = wk.tile([72, NCHUNK], f16, tag="u")
                # u = (dt * sb) then + Dp ; STT: (dt mult-bypass?)  -> use two ops
                nc.vector.scalar_tensor_tensor(
                    u[:, :], dtt, 0.0, sb[:, :], op0=ALU.add, op1=ALU.mult)
                v = wk.tile([72, NCHUNK], f16, tag="v")
                nc.gpsimd.tensor_tensor(v[:, :], xi_all[:, sl], sz_all[:, sl], op=ALU.mult)
                y2 = wk.tile([72, NCHUNK], f16, tag="y2")
                nc.vector.tensor_tensor(y2[:, :], v[:, :], u[:, :], op=ALU.mult)
                o32 = ps.tile([96, NCHUNK], f32, tag="pC")
                nc.tensor.matmul(o32[:, :], w["Lo"][:, :], y2[:, :], start=True, stop=False)
                nc.tensor.matmul(o32[:, :], w["LoD"][:, :], v[:, :], start=False, stop=True)
                e32 = wk.tile([96, NCHUNK], f16, tag="e32")
                nc.scalar.activation(e32[:, :], o32[:, :], AF.Exp, bias=w["b5t"][:, :], scale=1.0)
                sums_b = ps.tile([96, NCHUNK], f32, tag="pB")
                nc.tensor.matmul(sums_b[:, :], w["Lsum"][:, :], e32[:, :], start=True, stop=True)
                rb = wk.tile([96, NCHUNK], f32, tag="rb96", bufs=2)
                nc.vector.reciprocal_approx_fast(rb[:, :], sums_b[:, :])
                if c % 4 == 0:
                    nbs = min(4, NSB - c)
                    pr_big = wk.tile([96, nbs * NCHUNK], f16, tag="pr", bufs=2,
                                     name=f"pr_big_{c}")
                pr = pr_big[:, (c % 4) * NCHUNK:(c % 4 + 1) * NCHUNK]
                nc.gpsimd.tensor_tensor(pr[:, :], e32[:, :], rb[:, :], op=ALU.mult)
                if c % 4 == nbs - 1:
                    c0 = c - (c % 4)
                    nc.sync.dma_start(
                        outT[:, c0 * NCHUNK:(c0 + nbs) * NCHUNK], pr_big[:, :])
    nc.compile()
    return nc


def _get_program():
    global _PROGRAM
    if _PROGRAM is None:
        _PROGRAM = _build_program()
    return _PROGRAM


def kernel(**inputs) -> np.ndarray:
    from concourse.bass_utils import run_bass_kernel_spmd

    np_inputs = {k: np.asarray(v, np.float32) for k, v in inputs.items()}
    x = np_inputs.pop("x")
    weights = _fuse_weights(**np_inputs)

    in_maps = []
    for c in range(NCORES):
        xc = x[c * RPC:(c + 1) * RPC]
        xp = np.zeros((RPAD, 36), np.float32)
        xp[:RPC] = xc
        # row = g*NCOLS + n  ->  [G, NCOLS, 36] -> [G, 36, NCOLS] -> [108, NCOLS]
        xt = np.ascontiguousarray(
            xp.reshape(G, NCOLS, 36).transpose(0, 2, 1).reshape(108, NCOLS))
        xfull = np.ones((109, NCOLS), np.float32)
        xfull[:108] = xt
        in_maps.append({"xT": xfull.astype(np.float16), **weights})

    nc = _get_program()
    res = run_bass_kernel_spmd(nc, in_maps, core_ids=list(range(NCORES)), **_RUN_KW)
    global _LAST_RESULT
    _LAST_RESULT = res
    if getattr(res, "exec_time_ns", None):
        print(f"HW exec time: {res.exec_time_ns} ns")
    outs = []
    for c in range(NCORES):
        oT = np.asarray(res.results[c]["outT"], np.float32)   # [96, NCOLS]
        # partition g*32+f, col n -> row g*NCOLS+n, feature f
        o = oT.reshape(G, 32, NCOLS).transpose(0, 2, 1).reshape(RPAD, 32)
        outs.append(o[:RPC])
    return np.concatenate(outs, 0).astype(np.float32)


if __name__ == "__main__":
    nc = _build_program()
    print("program built OK")
